# revision 6
# baseline (speedup 1.0000x reference)
"""Trainium2 Bass kernel for BiologicalMultiHeadAttention.

Sharding (8 cores): core c -> (batch b = c//2, head-group g = c%2).
Each core computes, for its batch and its 8 heads (512 channels):
  q/k/v projections, dense softmax attention, neuromodulation gate,
  and a partial output projection over its 512 channels.
Host sums the two partial projections per batch and adds bo.

Every matmul is a 64-row PE tile (T0 = SBUF partitions 0-63, T8 =
64-127) so consecutive instructions on opposite row-halves stream
concurrently on the PE array (measured ~2x vs same-tile back-to-back)
and the tiling mode never switches:
  - scores: head pair (even head on partitions 0-63, odd on 64-127)
    computed as two concurrent K=64 matmuls into separate PSUM banks.
  - attn@V: per head K=128 over key positions, split into two K=64
    halves; the quad for a head pair is ordered (h0a:T0, h1a:T8,
    h1b:T0, h0b:T8) so no PSUM bank is written by two concurrent tiles.
  - projections / MLP / out-proj: contraction split into T0/T8 halves
    accumulated in two PSUM banks, combined in a 2-op DVE epilogue
    (DVE cannot read two PSUM operands in one instruction).
  - softmax denominators ride as an augmented ones-column of V; the
    reciprocal rows are pair-broadcast with a bf16 K=64 selector matmul
    (attn_scale folded into the selector).
Emission interleaves one projection/out-proj/normalize unit per
attention iteration so the ACT-engine exp backlog (the ~300us floor)
hides nearly all non-attention PE work.
"""

import numpy as np
import ml_dtypes

import concourse.bass as bass
import concourse.tile as tile
from concourse import bacc, mybir
from concourse.bass_utils import run_bass_kernel_spmd

F32 = mybir.dt.float32
BF16 = mybir.dt.bfloat16
AF = mybir.ActivationFunctionType
ALU = mybir.AluOpType

P = 128


def build_nc(S=2048, E=1024, HL=8, D=64, num_devices=8):
    CH = HL * D            # output channels per core (512)
    NE = E // P            # xT channel chunks (8)
    NC = CH // P           # qT/kT channel chunks == head pairs (4)
    NS = S // P            # key-position chunks (16)
    HM = E // 4            # mlp hidden (256)
    NH = HM // P           # h1T chunks (2)
    Q = 512                # query-block span
    NQB = S // Q           # query blocks (4)
    stW = Q // P           # rstage words per partition (4)
    HPC = P // D           # heads per channel chunk (2)
    scale = float(D) ** -0.5

    nc = bacc.Bacc("TRN2", target_bir_lowering=False, debug=False,
                   num_devices=num_devices)

    xT_d = nc.dram_tensor("xT", [E, S], BF16, kind="ExternalInput").ap()
    wqT_d = nc.dram_tensor("wqT", [E, CH], BF16, kind="ExternalInput").ap()
    wkT_d = nc.dram_tensor("wkT", [E, CH], BF16, kind="ExternalInput").ap()
    wvT_d = nc.dram_tensor("wvT", [E, CH], BF16, kind="ExternalInput").ap()
    wm1T_d = nc.dram_tensor("wm1T", [E, HM], BF16, kind="ExternalInput").ap()
    wm2T_d = nc.dram_tensor("wm2T", [HM, CH], BF16, kind="ExternalInput").ap()
    wo_d = nc.dram_tensor("wo", [CH, E], BF16, kind="ExternalInput").ap()
    bq_d = nc.dram_tensor("bq", [CH], F32, kind="ExternalInput").ap()
    bk_d = nc.dram_tensor("bk", [CH], F32, kind="ExternalInput").ap()
    bvr_d = nc.dram_tensor("bvr", [P, CH], F32, kind="ExternalInput").ap()
    bm1_d = nc.dram_tensor("bm1", [HM], F32, kind="ExternalInput").ap()
    bm2_d = nc.dram_tensor("bm2", [CH], F32, kind="ExternalInput").ap()
    # scal columns: dopamine, serotonin, norepinephrine, acetylcholine,
    # attn_scale, attn_bias, 0, 0 (replicated over 128 partitions)
    scal_d = nc.dram_tensor("scal", [P, 8], F32, kind="ExternalInput").ap()
    # selector rows 0/1 = attn_scale * one-hot(head parity), rows 2-63 = 0
    selp_d = nc.dram_tensor("selp", [D, P], BF16, kind="ExternalInput").ap()
    out_d = nc.dram_tensor("out", [S, E], F32, kind="ExternalOutput").ap()

    with tile.TileContext(nc) as tc:
        with (
            tc.tile_pool(name="const", bufs=1) as const,
            tc.tile_pool(name="xp", bufs=1) as xp,
            tc.tile_pool(name="expp", bufs=4) as expp,
            tc.tile_pool(name="tmpp", bufs=3) as tmpp,
            tc.tile_pool(name="evp", bufs=2) as evp,
            tc.tile_pool(name="denp", bufs=2) as denp,
            tc.tile_pool(name="outp", bufs=2) as outp,
            tc.tile_pool(name="scp", bufs=4, space="PSUM") as scp,
            tc.tile_pool(name="pjp", bufs=2, space="PSUM") as pjp,
            tc.tile_pool(name="accp", bufs=2, space="PSUM") as accp,
        ):
            # ---------------- loads (critical path first) ----------------
            xT = xp.tile([P, NE, S], BF16)
            for o in range(NE):
                nc.sync.dma_start(
                    xT[:, o, :],
                    xT_d.rearrange("(o p) f -> o p f", p=P)[o])

            wqT = xp.tile([P, NE, CH], BF16, tag="wqT")
            wkT = xp.tile([P, NE, CH], BF16, tag="wkT")
            # per-chunk column loads so chunk 0 lands first
            for m in range(NC):
                cs = slice(m * P, (m + 1) * P)
                nc.sync.dma_start(
                    wqT[:, :, cs],
                    wqT_d[:, cs].rearrange("(o p) f -> p o f", p=P))
                nc.sync.dma_start(
                    wkT[:, :, cs],
                    wkT_d[:, cs].rearrange("(o p) f -> p o f", p=P))

            def load_w(pool, dram, chunks, width, name):
                t = pool.tile([P, chunks, width], BF16, tag=name)
                nc.sync.dma_start(
                    t[:], dram.rearrange("(o p) f -> p o f", p=P))
                return t

            wvT = load_w(xp, wvT_d, NE, CH, "wvT")
            wm1T = load_w(xp, wm1T_d, NE, HM, "wm1T")
            wm2T = load_w(xp, wm2T_d, NH, CH, "wm2T")
            wo = load_w(const, wo_d, NC, E, "wo")

            def load_b(dram, chunks, name):
                t = const.tile([P, chunks], F32, tag=name)
                nc.sync.dma_start(t[:], dram.rearrange("(c p) -> p c", p=P))
                return t

            bq = load_b(bq_d, NC, "bq")
            bk = load_b(bk_d, NC, "bk")
            bm1 = load_b(bm1_d, NH, "bm1")
            bm2 = load_b(bm2_d, NC, "bm2")

            bv_bc = const.tile([P, CH], F32, tag="bv_bc")
            nc.sync.dma_start(bv_bc[:], bvr_d)

            selp = const.tile([D, P], BF16, tag="selp")
            nc.sync.dma_start(selp[:], selp_d)

            scal = const.tile([P, 8], F32, tag="scal")
            nc.sync.dma_start(scal[:], scal_d)

            # nm_gain = (dop + ser + nor + ace) / 4  -> [128, 1]
            nm = const.tile([P, 2], F32, tag="nm")
            nc.vector.tensor_tensor(nm[:, 0:1], scal[:, 0:1], scal[:, 1:2], ALU.add)
            nc.vector.tensor_tensor(nm[:, 1:2], scal[:, 2:3], scal[:, 3:4], ALU.add)
            nc.vector.tensor_tensor(nm[:, 0:1], nm[:, 0:1], nm[:, 1:2], ALU.add)
            nc.vector.tensor_scalar_mul(nm[:, 0:1], nm[:, 0:1], 0.25)
            nm_g = nm[:, 0:1]
            a_bias = scal[:, 5:6]

            # c1[m] = 1 + nm * bm2[m]: gate = psum*nm + c1
            c1 = const.tile([P, NC], F32, tag="c1")
            nc.vector.tensor_tensor(c1[:], bm2[:], nm_g.to_broadcast([P, NC]), ALU.mult)
            nc.vector.tensor_scalar_add(c1[:], c1[:], 1.0)

            # ---------------- persistent activations --------------------
            qT = const.tile([P, NC, S], BF16, tag="qT")
            kT = const.tile([P, NC, S], BF16, tag="kT")
            v_aug = const.tile([P, NS, HL, D + 1], BF16, tag="v_aug")
            h1T = const.tile([P, NH, S], BF16, tag="h1T")
            gateT = const.tile([P, NC, S], BF16, tag="gateT")
            attn_raw = const.tile([P, NC, S], BF16, tag="attn_raw")
            rstage = const.tile([P, HL, NQB, stW], F32, tag="rstage")
            rstage_b = const.tile([P, HL, NQB, stW], BF16, tag="rstage_b")
            # reciprocal-row staging for the selector matmul; rows 2-63
            # stay zero forever (selector rows are zero there, and
            # 0*garbage would still be NaN-unsafe)
            rd0 = const.tile([D, Q], BF16, tag="rd0")
            rd1 = const.tile([D, Q], BF16, tag="rd1")
            nc.vector.memset(rd0[:], 0.0)
            nc.vector.memset(rd1[:], 0.0)
            nc.vector.memset(v_aug[:, :, :, D:D + 1], 1.0)

            T0 = slice(0, D)
            T8 = slice(D, P)

            # -------- generic K-split projection group (T0/T8 pair) -----
            def mm_pair_group(wT, m, kchunks, src, col, psA, psB):
                for k in range(kchunks):
                    nc.tensor.matmul(
                        psA[:], wT[T0, k, m * P:(m + 1) * P], src[T0, k, col],
                        start=(k == 0), stop=(k == kchunks - 1))
                    nc.tensor.matmul(
                        psB[:], wT[T8, k, m * P:(m + 1) * P], src[T8, k, col],
                        start=(k == 0), stop=(k == kchunks - 1))

            def proj_unit(wT, dest, bias, m, kchunks, src, t4,
                          relu=False, gate=False, pool=None):
                pool = pool or pjp
                ptag = "sc" if pool is scp else "ps"
                col = slice(t4 * Q, (t4 + 1) * Q)
                psA = pool.tile([P, Q], F32, tag=ptag,
                                name=f"pA_{dest.tensor.name}_{m}_{t4}")
                psB = pool.tile([P, Q], F32, tag=ptag,
                                name=f"pB_{dest.tensor.name}_{m}_{t4}")
                mm_pair_group(wT, m, kchunks, src, col, psA, psB)
                t = tmpp.tile([P, Q], F32, tag="t",
                              name=f"t_{dest.tensor.name}_{m}_{t4}")
                if gate:
                    nc.vector.tensor_scalar(
                        t[:], psA[:], nm_g, c1[:, m:m + 1], ALU.mult, ALU.add)
                    nc.vector.scalar_tensor_tensor(
                        dest[:, m, col], psB[:], nm_g, t[:], ALU.mult, ALU.add)
                elif relu:
                    nc.vector.tensor_scalar_add(t[:], psA[:], bias[:, m:m + 1])
                    nc.vector.tensor_tensor(t[:], t[:], psB[:], ALU.add)
                    nc.scalar.activation(dest[:, m, col], t[:], AF.Relu)
                else:
                    nc.vector.tensor_scalar_add(t[:], psA[:], bias[:, m:m + 1])
                    nc.vector.tensor_tensor(dest[:, m, col], t[:], psB[:],
                                            ALU.add)

            def v_unit(c, pool=None):
                # v natural layout [kpos, ch] + ones column kept at col D
                pool = pool or pjp
                psA = pool.tile([P, Q], F32, tag="ps", name=f"vA_{c}")
                psB = pool.tile([P, Q], F32, tag="ps", name=f"vB_{c}")
                for k in range(NE):
                    nc.tensor.matmul(
                        psA[:, 0:CH], xT[T0, k, c * P:(c + 1) * P], wvT[T0, k, :],
                        start=(k == 0), stop=(k == NE - 1))
                    nc.tensor.matmul(
                        psB[:, 0:CH], xT[T8, k, c * P:(c + 1) * P], wvT[T8, k, :],
                        start=(k == 0), stop=(k == NE - 1))
                t = tmpp.tile([P, Q], F32, tag="t", name=f"vt_{c}")
                nc.vector.tensor_tensor(t[:, 0:CH], psA[:, 0:CH], bv_bc[:],
                                        ALU.add)
                nc.vector.tensor_tensor(
                    v_aug[:, c, :, 0:D],
                    t[:, 0:CH].rearrange("p (h d) -> p h d", h=HL),
                    psB[:, 0:CH].rearrange("p (h d) -> p h d", h=HL),
                    ALU.add)

            def outproj_unit(t4, n):
                rows = slice(t4 * P, (t4 + 1) * P)
                ncol = slice(n * Q, (n + 1) * Q)
                psA = pjp.tile([P, Q], F32, tag="ps", name=f"oA_{t4}_{n}")
                psB = pjp.tile([P, Q], F32, tag="ps", name=f"oB_{t4}_{n}")
                for k in range(NC):
                    nc.tensor.matmul(
                        psA[:], attn_raw[T0, k, rows], wo[T0, k, ncol],
                        start=(k == 0), stop=(k == NC - 1))
                    nc.tensor.matmul(
                        psB[:], attn_raw[T8, k, rows], wo[T8, k, ncol],
                        start=(k == 0), stop=(k == NC - 1))
                ot = outp.tile([P, Q], F32, tag="osb", name=f"osb_{t4}_{n}")
                nc.vector.tensor_copy(ot[:], psA[:])
                nc.vector.tensor_tensor(ot[:], ot[:], psB[:], ALU.add)
                nc.sync.dma_start(out_d[rows, ncol], ot[:])

            def tail_unit(pr, qb):
                # bc[p, q] = attn_scale / den[head(p), q] via selector mm,
                # then final = (raw * bc + attn_bias) * gate, in place
                rd = rd0 if (pr + qb) % 2 == 0 else rd1
                for hp in range(HPC):
                    nc.sync.dma_start(
                        rd[hp:hp + 1, :],
                        rstage_b[:, pr * HPC + hp, qb, :])
                col = slice(qb * Q, (qb + 1) * Q)
                bcp = pjp.tile([P, Q], F32, tag="ps", name=f"bc_{pr}_{qb}")
                nc.tensor.matmul(bcp[:], selp[:], rd[:], start=True, stop=True)
                t = tmpp.tile([P, Q], F32, tag="t", name=f"tl_{pr}_{qb}")
                nc.vector.tensor_tensor(t[:], attn_raw[:, pr, col], bcp[:],
                                        ALU.mult)
                nc.vector.scalar_tensor_tensor(
                    attn_raw[:, pr, col], t[:], a_bias, gateT[:, pr, col],
                    ALU.add, ALU.mult)

            # ---------------- work queue ----------------
            from collections import deque
            pending = deque()

            def drip():
                if pending:
                    pending.popleft()()

            def drain():
                while pending:
                    pending.popleft()()

            def queue_proj_chunk(wT, dest, bias, m, kchunks, src,
                                 relu=False, gate=False):
                for t4 in range(S // Q):
                    pending.append(
                        lambda t4=t4: proj_unit(wT, dest, bias, m, kchunks,
                                                src, t4, relu, gate))

            # ---------------- attention ----------------
            def attn_iter(pair, qb, j, accs, inline=None):
                qcol = slice(qb * Q, (qb + 1) * Q)
                h0, h1 = pair * HPC, pair * HPC + 1
                if inline is not None:
                    inline()
                sc0 = scp.tile([P, Q], F32, tag="sc", name=f"s0_{pair}_{qb}_{j}")
                sc1 = scp.tile([P, Q], F32, tag="sc", name=f"s1_{pair}_{qb}_{j}")
                jrow = slice(j * P, (j + 1) * P)
                nc.tensor.matmul(sc0[:], kT[T0, pair, jrow], qT[T0, pair, qcol],
                                 start=True, stop=True)
                nc.tensor.matmul(sc1[:], kT[T8, pair, jrow], qT[T8, pair, qcol],
                                 start=True, stop=True)
                ex0 = expp.tile([P, Q], BF16, tag="ex", name=f"e0_{pair}_{qb}_{j}")
                ex1 = expp.tile([P, Q], BF16, tag="ex", name=f"e1_{pair}_{qb}_{j}")
                nc.scalar.activation(ex0[:], sc0[:], AF.Exp, scale=scale)
                nc.scalar.activation(ex1[:], sc1[:], AF.Exp, scale=scale)
                acc0, acc1 = accs
                st, sp = j == 0, j == NS - 1
                # K=128 full-row AV (bisect variant)
                nc.tensor.matmul(acc0[:], v_aug[:, j, h0, :], ex0[:],
                                 start=st, stop=sp)
                nc.tensor.matmul(acc1[:], v_aug[:, j, h1, :], ex1[:],
                                 start=st, stop=sp)
                if inline is None:
                    drip()

            def attn_sweep(pair, qb, first=False):
                accs = (
                    accp.tile([D + 1, Q], F32, tag="acc", name=f"a0_{pair}_{qb}"),
                    accp.tile([D + 1, Q], F32, tag="acc", name=f"a1_{pair}_{qb}"),
                )
                for j in range(NS):
                    inline = (lambda j=j: v_unit(j)) if first else None
                    attn_iter(pair, qb, j, accs, inline=inline)
                # evict raw attention (bf16) + denominator rows
                for hp in range(HPC):
                    h = pair * HPC + hp
                    acc = accs[hp]
                    tmp = evp.tile([D, Q], BF16, tag="ev", name=f"ev_{h}_{qb}")
                    nc.vector.tensor_copy(tmp[:], acc[0:D, :])
                    den = denp.tile([1, Q], F32, tag="den", name=f"dn_{h}_{qb}")
                    nc.vector.tensor_copy(den[:], acc[D:D + 1, :])
                    nc.sync.dma_start(
                        attn_raw[hp * D:(hp + 1) * D, pair,
                                 qb * Q:(qb + 1) * Q],
                        tmp[:])
                    nc.sync.dma_start(rstage[:, h, qb, :], den[:])
                with nc.allow_low_precision(
                        reason="bf16 softmax reciprocal rows"):
                    nc.vector.reciprocal(
                        rstage_b[:, pair * HPC:(pair + 1) * HPC, qb, :],
                        rstage[:, pair * HPC:(pair + 1) * HPC, qb, :])

            # ---------------- emission schedule ----------------
            assert HL == 8 and NC == 4 and NQB == 4

            # prologue: q/k chunk 0 directly, pipelined across both pools
            for t4 in range(S // Q):
                proj_unit(wqT, qT, bq, 0, NE, xT, t4, pool=(scp, pjp)[t4 % 2])
                proj_unit(wkT, kT, bk, 0, NE, xT, t4, pool=(pjp, scp)[t4 % 2])

            # half 0 (query blocks 0-1): pair-outer so v/qk projections
            # have a long window to hide in
            for pair in range(NC):
                if pair < NC - 1:
                    queue_proj_chunk(wqT, qT, bq, pair + 1, NE, xT)
                    queue_proj_chunk(wkT, kT, bk, pair + 1, NE, xT)
                if pair == 1:
                    for m in range(NH):
                        queue_proj_chunk(wm1T, h1T, bm1, m, NE, xT, relu=True)
                if pair == 2:
                    for m in range(NC):
                        queue_proj_chunk(wm2T, gateT, bm2, m, NH, h1T, gate=True)
                for qb in range(2):
                    attn_sweep(pair, qb, first=(pair == 0 and qb == 0))

            # tails + out-projection for half 0, dripped into half 1
            for qb in range(2):
                for pr in range(NC):
                    pending.append(lambda pr=pr, qb=qb: tail_unit(pr, qb))
            for t4 in range(S // P // 2):
                for n in range(E // Q):
                    pending.append(lambda t4=t4, n=n: outproj_unit(t4, n))

            # half 1 (query blocks 2-3): qb-outer so qb2's tails/out-proj
            # drip into qb3's attention
            for qb in range(2, 4):
                for pair in range(NC):
                    attn_sweep(pair, qb)
                if qb == 2:
                    for pr in range(NC):
                        pending.append(lambda pr=pr: tail_unit(pr, 2))
                    for t4 in range(8, 12):
                        for n in range(E // Q):
                            pending.append(
                                lambda t4=t4, n=n: outproj_unit(t4, n))
            drain()
            for pr in range(NC):
                tail_unit(pr, 3)
            for t4 in range(12, 16):
                for n in range(E // Q):
                    outproj_unit(t4, n)

    nc.compile()
    return nc


_CACHE = {}


def _get_nc():
    if "nc" not in _CACHE:
        _CACHE["nc"] = build_nc()
    return _CACHE["nc"]


def _bf16_t(a):
    """transpose + cast to contiguous bf16"""
    return np.ascontiguousarray(np.asarray(a, np.float32).T).astype(ml_dtypes.bfloat16)


def kernel(query, Wq, bq, Wk, bk, Wv, bv, Wo, bo,
           Wm1, bm1, Wm2, bm2,
           dopamine, serotonin, norepinephrine, acetylcholine,
           attn_scale, attn_bias):
    B, S, E = 4, 2048, 1024
    CH = 512
    D_ = 64
    nc = _get_nc()

    query = np.asarray(query, np.float32)
    f32 = lambda a: np.ascontiguousarray(np.asarray(a, np.float32))
    a_scale = float(np.asarray(attn_scale).reshape(-1)[0])
    scal_row = np.array([float(np.asarray(dopamine).reshape(-1)[0]),
                         float(np.asarray(serotonin).reshape(-1)[0]),
                         float(np.asarray(norepinephrine).reshape(-1)[0]),
                         float(np.asarray(acetylcholine).reshape(-1)[0]),
                         a_scale,
                         float(np.asarray(attn_bias).reshape(-1)[0]),
                         0.0, 0.0], np.float32)
    scal = np.tile(scal_row[None, :], (128, 1))
    selp = np.zeros((D_, 128), np.float32)
    selp[0, 0:D_] = a_scale
    selp[1, D_:2 * D_] = a_scale
    selp = selp.astype(ml_dtypes.bfloat16)

    wm1T = _bf16_t(Wm1)
    in_maps = []
    for core in range(8):
        b, g = core // 2, core % 2
        cg = slice(g * CH, (g + 1) * CH)
        Wo_np = np.asarray(Wo, np.float32)
        in_maps.append({
            "xT": _bf16_t(query[b]),
            "wqT": _bf16_t(np.asarray(Wq, np.float32)[cg]),
            "wkT": _bf16_t(np.asarray(Wk, np.float32)[cg]),
            "wvT": _bf16_t(np.asarray(Wv, np.float32)[cg]),
            "wm1T": wm1T,
            "wm2T": _bf16_t(np.asarray(Wm2, np.float32)[cg]),
            "wo": _bf16_t(Wo_np[:, cg]),
            "bq": f32(np.asarray(bq, np.float32)[cg]),
            "bk": f32(np.asarray(bk, np.float32)[cg]),
            "bvr": np.ascontiguousarray(
                np.tile(np.asarray(bv, np.float32)[cg][None, :], (128, 1))),
            "bm1": f32(bm1),
            "bm2": f32(np.asarray(bm2, np.float32)[cg]),
            "scal": scal,
            "selp": selp,
        })

    res = run_bass_kernel_spmd(nc, in_maps, core_ids=list(range(8)))
    _CACHE["last_results"] = res

    bo_np = np.asarray(bo, np.float32)
    out = np.empty((B, S, E), np.float32)
    for b in range(B):
        out[b] = res.results[2 * b]["out"] + res.results[2 * b + 1]["out"] + bo_np
    return out
